# revision 1
# baseline (speedup 1.0000x reference)
"""Trainium2 Bass kernel for the DEER-MLP spiking network.

Network: x(4,32,196,384) -> FC1(384->1536) -> BatchNorm -> LIF(T=4) ->
FC2(1536->384) -> BatchNorm -> LIF -> spikes(4,32,196,384).

Math note: the reference solves the LIF recurrence with 10 DEER Newton
iterations over T=4 steps. Newton on a length-T triangular system is exact
after T iterations, so the converged result equals the plain sequential
recurrence; we compute that directly (4 elementwise steps).

Distribution: data-parallel over the flattened B*N batch across 8 cores
(784 lanes/core). BatchNorm statistics are the only cross-core coupling:
two tiny AllReduces ([128,24] and [128,6] fp32).

Precision: both matmuls run as multi-pass fp16 with operands split into
hi/lo fp16 limbs (split on host; the PE honors fp16 subnormals, verified
on hardware). fp16 products accumulate exactly into fp32 PSUM, so
FC1 = x_hi@w_hi + x_lo@w_hi + x_hi@w_lo reproduces fp32 to ~2^-22 (the
dropped lo@lo term), and FC2's spikes are exactly 0/1 in fp16 so two
passes (w_hi + w_lo) are ~2^-22 as well. This is 4x (FC2) / 1.33x (FC1)
faster than native fp32 matmul on the PE at fp32-level accuracy.

Per-core pipeline (single NEFF):
  A: FC1 on PE; bias add + per-channel sum/sumsq fused into the PSUM
     evacuation on the Scalar engine (accum_out); y1 -> DRAM scratch.
     AllReduce BN1 stats.
  B: BN1 affine + 4-step LIF on DVE; spikes stored fp16; FC2 fp16;
     BN2 stats fused in evacuation; y2 kept SBUF-resident. AllReduce.
  C: BN2 affine + LIF on DVE in place.
  D: PE-transpose spikes back to row-major, DMA out.

Host-side prep in kernel(): shard x over B, pre-transpose to [C, R] and
split into fp16 limbs; pre-transpose W1/W2 and split into fp16 limbs.
"""

import numpy as np

import concourse.bass as bass
import concourse.mybir as mybir
import concourse.tile as tile
from concourse import bacc
from concourse.bass_utils import run_bass_kernel_spmd
F32 = mybir.dt.float32
F16 = mybir.dt.float16
AF = mybir.ActivationFunctionType
OP = mybir.AluOpType
AX = mybir.AxisListType

T, B, NN, C, H = 4, 32, 196, 384, 1536
NCORES = 8
BLOC = B // NCORES            # 4 batches per core
MLOC = BLOC * NN              # 784 lanes per core
R = T * MLOC                  # 3136 flattened (t, m) rows per core
NTOT = T * B * NN             # 25088 batchnorm samples per channel
KC = C // 128                 # 3 c-tiles
KH = H // 128                 # 12 h-tiles
EPS = 1e-5
P = 128

A_CHUNKS = [(i * 512, 512) for i in range(R // 512)] + [(R - R % 512, R % 512)]
B_CHUNKS = [(0, MLOC // 2), (MLOC // 2, MLOC // 2)]


def _lif(nc, pool, drive, s_out, mlen, tag):
    """Sequential LIF over T steps.

    drive: [128, T, >=mlen] fp32 (already 0.5*BN(y)); s_out: [128, T, >=mlen]
    spike output. h_t = 0.5*v_{t-1} + drive_t; s = (h>=1); v = h*(h<1).
    s_out may alias drive (phase C writes spikes in place).
    """
    v = pool.tile([P, mlen], F32, tag=f"{tag}_v", name=f"{tag}_v")
    nc.vector.scalar_tensor_tensor(
        v[:], drive[:, 0, :mlen], 1.0, drive[:, 0, :mlen], OP.is_lt, OP.mult
    )
    nc.vector.tensor_scalar(
        s_out[:, 0, :mlen], drive[:, 0, :mlen], 1.0, None, OP.is_ge
    )
    for t in range(1, T):
        h = pool.tile([P, mlen], F32, tag=f"{tag}_h", name=f"{tag}_h")
        nc.vector.scalar_tensor_tensor(
            h[:], v[:], 0.5, drive[:, t, :mlen], OP.mult, OP.add
        )
        if t < T - 1:
            v = pool.tile([P, mlen], F32, tag=f"{tag}_v", name=f"{tag}_v")
            nc.vector.scalar_tensor_tensor(v[:], h[:], 1.0, h[:], OP.is_lt, OP.mult)
        nc.vector.tensor_scalar(s_out[:, t, :mlen], h[:], 1.0, None, OP.is_ge)


def _bn_coeffs(nc, pool, stg, gt, bet, k, tag):
    """From allreduced [128, 2k] (sum || sumsq) compute the fused affine
    drive = y*dsc + dsh  ==  0.5 * ((y - mean) * rsqrt(var+eps) * g + be)."""
    mean = pool.tile([P, k], F32, tag=f"{tag}_mean", name=f"{tag}_mean")
    nc.vector.tensor_scalar(mean[:], stg[:, 0:k], 1.0 / NTOT, None, OP.mult)
    var = pool.tile([P, k], F32, tag=f"{tag}_var", name=f"{tag}_var")
    nc.vector.tensor_scalar(var[:], stg[:, k : 2 * k], 1.0 / NTOT, None, OP.mult)
    msq = pool.tile([P, k], F32, tag=f"{tag}_msq", name=f"{tag}_msq")
    nc.vector.tensor_tensor(msq[:], mean[:], mean[:], OP.mult)
    nc.vector.tensor_tensor(var[:], var[:], msq[:], OP.subtract)
    nc.vector.tensor_scalar(var[:], var[:], EPS, None, OP.add)
    std = pool.tile([P, k], F32, tag=f"{tag}_std", name=f"{tag}_std")
    nc.scalar.activation(std[:], var[:], AF.Sqrt, bias=0.0, scale=1.0)
    rstd = pool.tile([P, k], F32, tag=f"{tag}_rstd", name=f"{tag}_rstd")
    nc.vector.reciprocal(rstd[:], std[:])
    dsc = pool.tile([P, k], F32, tag=f"{tag}_dsc", name=f"{tag}_dsc")
    nc.vector.tensor_tensor(dsc[:], rstd[:], gt[:], OP.mult)
    dsh = pool.tile([P, k], F32, tag=f"{tag}_dsh", name=f"{tag}_dsh")
    nc.vector.tensor_tensor(dsh[:], mean[:], dsc[:], OP.mult)
    nc.vector.tensor_tensor(dsh[:], bet[:], dsh[:], OP.subtract)
    nc.vector.tensor_scalar(dsc[:], dsc[:], 0.5, None, OP.mult)
    nc.vector.tensor_scalar(dsh[:], dsh[:], 0.5, None, OP.mult)
    return dsc, dsh


def _build():
    nc = bacc.Bacc("TRN2", target_bir_lowering=False, debug=False,
                   num_devices=NCORES)

    xh_d = nc.dram_tensor("xthi", [KC, P, R], F16, kind="ExternalInput")
    xl_d = nc.dram_tensor("xtlo", [KC, P, R], F16, kind="ExternalInput")
    w1h_d = nc.dram_tensor("w1thi", [KC, P, H], F16, kind="ExternalInput")
    w1l_d = nc.dram_tensor("w1tlo", [KC, P, H], F16, kind="ExternalInput")
    w2h_d = nc.dram_tensor("w2thi", [KH, P, C], F16, kind="ExternalInput")
    w2l_d = nc.dram_tensor("w2tlo", [KH, P, C], F16, kind="ExternalInput")
    b1_d = nc.dram_tensor("b1", [H], F32, kind="ExternalInput")
    g1_d = nc.dram_tensor("g1", [H], F32, kind="ExternalInput")
    be1_d = nc.dram_tensor("be1", [H], F32, kind="ExternalInput")
    b2_d = nc.dram_tensor("b2", [C], F32, kind="ExternalInput")
    g2_d = nc.dram_tensor("g2", [C], F32, kind="ExternalInput")
    be2_d = nc.dram_tensor("be2", [C], F32, kind="ExternalInput")
    out_d = nc.dram_tensor("out", [R, C], F32, kind="ExternalOutput")

    groups = [list(range(NCORES))]

    with tile.TileContext(nc) as tc:
        with (
            tc.tile_pool(name="const", bufs=1) as const,
            tc.tile_pool(name="dram", bufs=1, space="DRAM") as dram,
        ):
            def colvec(dst_k, src):
                t_ = const.tile([P, dst_k], F32, name=f"cv_{src.name}",
                                tag=f"cv_{src.name}")
                nc.sync.dma_start(
                    t_[:], src.ap().rearrange("(a p) -> p a", p=P)
                )
                return t_

            b1t, g1t, be1t = (colvec(KH, d) for d in (b1_d, g1_d, be1_d))
            b2t, g2t, be2t = (colvec(KC, d) for d in (b2_d, g2_d, be2_d))

            w2h = const.tile([P, KH, C], F16)
            nc.sync.dma_start(w2h[:], w2h_d.ap().rearrange("k p c -> p k c"))
            w2l = const.tile([P, KH, C], F16)
            nc.sync.dma_start(w2l[:], w2l_d.ap().rearrange("k p c -> p k c"))

            # --- phase A: FC1 (3-pass bf16) + BN1 partial stats ---------
            y1s = dram.tile([KH, P, R], F32)
            asum1 = const.tile([P, KH, len(A_CHUNKS)], F32)
            asq1 = const.tile([P, KH, len(A_CHUNKS)], F32)
            with (
                tc.tile_pool(name="pax", bufs=1) as pax,
                tc.tile_pool(name="pa", bufs=4) as pa,
                tc.tile_pool(name="ps_mm", bufs=6, space="PSUM") as ps_mm,
            ):
                w1h = pax.tile([P, KC, H], F16)
                nc.sync.dma_start(w1h[:], w1h_d.ap().rearrange("k p h -> p k h"))
                w1l = pax.tile([P, KC, H], F16)
                nc.sync.dma_start(w1l[:], w1l_d.ap().rearrange("k p h -> p k h"))
                xh = pax.tile([P, KC, R], F16)
                nc.sync.dma_start(xh[:], xh_d.ap().rearrange("k p r -> p k r"))
                xl = pax.tile([P, KC, R], F16)
                nc.sync.dma_start(xl[:], xl_d.ap().rearrange("k p r -> p k r"))

                for ci, (r0, rlen) in enumerate(A_CHUNKS):
                    for a in range(KH):
                        ps = ps_mm.tile([P, 512], F32, tag="mm")
                        idx = 0
                        for wt, xt in ((w1h, xh), (w1l, xh), (w1h, xl)):
                            for k in range(KC):
                                nc.tensor.matmul(
                                    ps[:, :rlen],
                                    wt[:, k, a * P : (a + 1) * P],
                                    xt[:, k, r0 : r0 + rlen],
                                    start=(idx == 0),
                                    stop=(idx == 8),
                                )
                                idx += 1
                        y1sb = pa.tile([P, 512], F32, tag="y1sb")
                        nc.scalar.activation(
                            y1sb[:, :rlen], ps[:, :rlen], AF.Identity,
                            bias=b1t[:, a : a + 1], scale=1.0,
                            accum_out=asum1[:, a, ci : ci + 1],
                        )
                        sqt = pa.tile([P, 512], F32, tag="sqt")
                        nc.scalar.activation(
                            sqt[:, :rlen], ps[:, :rlen], AF.Square,
                            bias=b1t[:, a : a + 1], scale=1.0,
                            accum_out=asq1[:, a, ci : ci + 1],
                        )
                        nc.sync.dma_start(y1s[a, :, r0 : r0 + rlen],
                                          y1sb[:, :rlen])

            # --- BN1 stat allreduce -------------------------------------
            # Stats DMAs ride the gpsimd queue: a collective-gated load at
            # the head of the sync queue would head-of-line block the
            # phase-B y1 prefetch below.
            st1 = const.tile([P, 2 * KH], F32)
            nc.vector.tensor_reduce(st1[:, 0:KH], asum1[:], AX.X, OP.add)
            nc.vector.tensor_reduce(st1[:, KH : 2 * KH], asq1[:], AX.X, OP.add)
            st1_in = dram.tile([P, 2 * KH], F32)
            st1_out = dram.tile([P, 2 * KH], F32)
            nc.gpsimd.dma_start(st1_in[:], st1[:])
            nc.gpsimd.collective_compute(
                "AllReduce", OP.add, replica_groups=groups,
                ins=[st1_in.opt()], outs=[st1_out.opt()],
            )
            stg1 = const.tile([P, 2 * KH], F32)
            nc.gpsimd.dma_start(stg1[:], st1_out[:])
            dsc1, dsh1 = _bn_coeffs(nc, const, stg1, g1t, be1t, KH, "bn1")

            # --- phase B: BN1 + LIF1 + FC2 (2-pass fp16) + BN2 stats ----
            y2r = [const.tile([P, T, MLOC], F32, tag=f"y2r{ct}",
                              name=f"y2r{ct}")
                   for ct in range(KC)]
            nb2 = len(B_CHUNKS) * T
            asum2 = const.tile([P, KC, nb2], F32)
            asq2 = const.tile([P, KC, nb2], F32)
            with (
                tc.tile_pool(name="pb", bufs=4) as pb,
                tc.tile_pool(name="pb_s1", bufs=13) as pbs1,
                tc.tile_pool(name="ps_mm2", bufs=4, space="PSUM") as ps_mm2,
            ):
                # Prefetch the first half of chunk-0 y1 while the stat
                # allreduce is in flight (the loads depend only on phase-A
                # scratch writes, not on the collective).
                NPRE = 6
                yt_pre = []
                m0p, mlenp = B_CHUNKS[0]
                for a in range(NPRE):
                    yt = pb.tile([P, T, mlenp], F32, tag="yt", bufs=8,
                                 name=f"yt_pre{a}")
                    src = y1s[a].rearrange("p (t m) -> p t m", t=T)
                    nc.sync.dma_start(yt[:], src[:, :, m0p : m0p + mlenp])
                    yt_pre.append(yt)
                for mi, (m0, mlen) in enumerate(B_CHUNKS):
                    s1_tiles = []
                    for a in range(KH):
                        if mi == 0 and a < NPRE:
                            yt = yt_pre[a]
                        else:
                            yt = pb.tile([P, T, mlen], F32, tag="yt", bufs=8,
                                         name=f"yt{mi}_{a}")
                            src = y1s[a].rearrange("p (t m) -> p t m", t=T)
                            nc.sync.dma_start(yt[:], src[:, :, m0 : m0 + mlen])
                        nc.vector.tensor_scalar(
                            yt[:], yt[:], dsc1[:, a : a + 1],
                            dsh1[:, a : a + 1], OP.mult, OP.add,
                        )
                        st_ = pbs1.tile([P, T, mlen], F16, tag="s1")
                        _lif(nc, pb, yt, st_, mlen, "lif1")
                        s1_tiles.append(st_)
                    for t in range(T):
                        for ct in range(KC):
                            ps2 = ps_mm2.tile([P, 512], F32, tag="mm2")
                            idx = 0
                            for k in range(KH):
                                for wsp in (w2h, w2l):
                                    nc.tensor.matmul(
                                        ps2[:, :mlen],
                                        wsp[:, k, ct * P : (ct + 1) * P],
                                        s1_tiles[k][:, t, :mlen],
                                        start=(idx == 0),
                                        stop=(idx == 2 * KH - 1),
                                    )
                                    idx += 1
                            ci2 = mi * T + t
                            nc.scalar.activation(
                                y2r[ct][:, t, m0 : m0 + mlen], ps2[:, :mlen],
                                AF.Identity, bias=b2t[:, ct : ct + 1],
                                scale=1.0,
                                accum_out=asum2[:, ct, ci2 : ci2 + 1],
                            )
                            sqt2 = pb.tile([P, 512], F32, tag="sqt2")
                            nc.scalar.activation(
                                sqt2[:, :mlen], ps2[:, :mlen], AF.Square,
                                bias=b2t[:, ct : ct + 1], scale=1.0,
                                accum_out=asq2[:, ct, ci2 : ci2 + 1],
                            )

            # --- BN2 stat allreduce -------------------------------------
            st2 = const.tile([P, 2 * KC], F32)
            nc.vector.tensor_reduce(st2[:, 0:KC], asum2[:], AX.X, OP.add)
            nc.vector.tensor_reduce(st2[:, KC : 2 * KC], asq2[:], AX.X, OP.add)
            st2_in = dram.tile([P, 2 * KC], F32)
            st2_out = dram.tile([P, 2 * KC], F32)
            nc.gpsimd.dma_start(st2_in[:], st2[:])
            nc.gpsimd.collective_compute(
                "AllReduce", OP.add, replica_groups=groups,
                ins=[st2_in.opt()], outs=[st2_out.opt()],
            )
            stg2 = const.tile([P, 2 * KC], F32)
            nc.gpsimd.dma_start(stg2[:], st2_out[:])
            dsc2, dsh2 = _bn_coeffs(nc, const, stg2, g2t, be2t, KC, "bn2")

            # --- phase C: BN2 (in place) + LIF2 -> fp16 spikes ----------
            RPAD = ((R + P - 1) // P) * P
            with (
                tc.tile_pool(name="ps2", bufs=1) as ps2,
                tc.tile_pool(name="pc", bufs=3) as pc,
                tc.tile_pool(name="pd", bufs=4) as pd,
            ):
                s2t = [ps2.tile([P, RPAD], F16, tag=f"s2t{ct}",
                                name=f"s2t{ct}")
                       for ct in range(KC)]
                for ct in range(KC):
                    if RPAD > R:
                        nc.vector.memset(s2t[ct][:, R:], 0.0)
                    nc.vector.tensor_scalar(
                        y2r[ct][:], y2r[ct][:], dsc2[:, ct : ct + 1],
                        dsh2[:, ct : ct + 1], OP.mult, OP.add,
                    )
                    s2v = s2t[ct][:, :R].rearrange("p (t m) -> p t m", t=T)
                    _lif(nc, pc, y2r[ct], s2v, MLOC, "lif2")

                # --- phase D: DMA-transpose (XBAR, fp16) + upcast -------
                for r0 in range(0, R, P):
                    rlen = min(P, R - r0)
                    ob16 = pd.tile([P, C], F16, tag="ob16")
                    for ct in range(KC):
                        nc.sync.dma_start_transpose(
                            ob16[:, ct * P : (ct + 1) * P],
                            s2t[ct][:, r0 : r0 + P],
                        )
                    ob = pd.tile([P, C], F32, tag="ob")
                    nc.vector.tensor_copy(ob[:rlen], ob16[:rlen])
                    nc.sync.dma_start(out_d[r0 : r0 + rlen, :], ob[:rlen, :])


    nc.compile()
    return nc


_NC = None
TRACE = False          # set by test harness to capture an NTFF profile
LAST_RESULT = None     # BassKernelResults of the most recent run


def _get_nc():
    global _NC
    if _NC is None:
        _NC = _build()
    return _NC


def _split_f16(a):
    hi = a.astype(np.float16)
    lo = (a - hi.astype(np.float32)).astype(np.float16)
    return np.ascontiguousarray(hi), np.ascontiguousarray(lo)


def _in_maps(x, W1, b1, g1, be1, W2, b2, g2, be2):
    x = np.asarray(x, dtype=np.float32)
    w1t = np.asarray(W1, np.float32).T.reshape(KC, P, H)
    w1thi, w1tlo = _split_f16(w1t)
    w2t = np.asarray(W2, np.float32).T.reshape(KH, P, C)
    w2thi, w2tlo = _split_f16(w2t)
    shared = {
        "w1thi": w1thi, "w1tlo": w1tlo,
        "w2thi": w2thi, "w2tlo": w2tlo,
        "b1": np.asarray(b1, np.float32),
        "g1": np.asarray(g1, np.float32),
        "be1": np.asarray(be1, np.float32),
        "b2": np.asarray(b2, np.float32),
        "g2": np.asarray(g2, np.float32),
        "be2": np.asarray(be2, np.float32),
    }
    in_maps = []
    for i in range(NCORES):
        xt = x[:, i * BLOC : (i + 1) * BLOC].reshape(R, C).T.reshape(KC, P, R)
        xthi, xtlo = _split_f16(xt)
        in_maps.append({"xthi": xthi, "xtlo": xtlo, **shared})
    return in_maps


def kernel(x, W1, b1, g1, be1, W2, b2, g2, be2):
    nc = _get_nc()
    in_maps = _in_maps(x, W1, b1, g1, be1, W2, b2, g2, be2)
    res = run_bass_kernel_spmd(nc, in_maps, core_ids=list(range(NCORES)),
                               trace=TRACE)
    global LAST_RESULT
    LAST_RESULT = res
    out = np.concatenate(
        [res.results[i]["out"].reshape(T, BLOC, NN, C) for i in range(NCORES)],
        axis=1,
    )
    return out



# revision 8
# speedup vs baseline: 1.2563x; 1.2563x over previous
"""Trainium2 Bass kernel for the DEER-MLP spiking network (v2).

Network: x(4,32,196,384) -> FC1(384->1536) -> BatchNorm -> LIF(T=4) ->
FC2(1536->384) -> BatchNorm -> LIF -> spikes(4,32,196,384).

Math notes:
- 10 DEER Newton iterations over T=4 steps converge exactly to the plain
  sequential LIF recurrence; we compute that directly.
- b1/b2 are dropped: BatchNorm immediately follows each Linear and is
  invariant to per-channel shifts.
- Spikes are produced as s' = Sign(h-1) in {-1,+1} by the Scalar engine.
  FC2 runs directly on s' : y2' = W2 @ s' = 2*y2 - rowsum(W2), and BN2 is
  invariant to per-channel affine maps, so the final result is unchanged.
  The host maps the final spike train back to {0,1} via (s'+1)/2.

Distribution: data-parallel over B across 8 cores (4 batches / 784 lanes
per core).  BatchNorm statistics are the only cross-core coupling.

Pipeline (single NEFF per core):
- FC1 runs htile-outer (12 tiles of 128 H-channels x 7 R-chunks, 3-pass
  fp16 limb matmul).  Each htile's y1 stays SBUF-resident; PSUM is
  evacuated by the Scalar engine (Identity+accum for sum, Square+accum
  for sumsq, the square written back into PSUM in place).
- Per-htile [128,2] stat AllReduce rides the gpsimd queue, pipelined
  under the FC1 matmuls of later htiles (no global barrier).
- LIF1 per htile (lagged 2 htiles behind FC1): BN affine as one DVE
  tensor_scalar, h/v updates as 6 DVE scalar_tensor_tensor ops, spikes
  as one Scalar Sign activation into fp16 s1.
- FC2 (2-pass fp16) consumes s1; stats fused into PSUM evacuation; one
  [128,6] AllReduce; LIF2 on DVE; Sign + DMA out per timestep, fp16.

Host side: pre-transpose/limb-split x and W; post-transpose the fp16
+-1 spike output back to (T,B,N,C) fp32 {0,1}.
"""

import numpy as np

import concourse.bass as bass
import concourse.mybir as mybir
import concourse.tile as tile
from concourse import bacc
from concourse.bass_utils import run_bass_kernel_spmd

F32 = mybir.dt.float32
F16 = mybir.dt.float16
AF = mybir.ActivationFunctionType
OP = mybir.AluOpType
AX = mybir.AxisListType

T, B, NN, C, H = 4, 32, 196, 384, 1536
NCORES = 8
BLOC = B // NCORES            # 4 batches per core
MLOC = BLOC * NN              # 784 lanes per core
R = T * MLOC                  # 3136 flattened (t, m) rows per core
NTOT = T * B * NN             # 25088 batchnorm samples per channel
KC = C // 128                 # 3 c-tiles
KH = H // 128                 # 12 h-tiles
EPS = 1e-5
P = 128
LAG = 4                       # htiles between FC1 and its LIF emission

# FC1 R-chunk pairs: PSUM tiles of [128, 1024] (2 banks), last 64 ragged.
A_PAIRS = [(0, 1024), (1024, 1024), (2048, 1024), (3072, 64)]
B_CHUNKS = [(0, MLOC // 2), (MLOC // 2, MLOC // 2)]


def _build():
    nc = bacc.Bacc("TRN2", target_bir_lowering=False, debug=False,
                   num_devices=NCORES)

    xh_d = nc.dram_tensor("xthi", [KC, P, R], F16, kind="ExternalInput")
    xl_d = nc.dram_tensor("xtlo", [KC, P, R], F16, kind="ExternalInput")
    w1h_d = nc.dram_tensor("w1thi", [KC, P, H], F16, kind="ExternalInput")
    w1l_d = nc.dram_tensor("w1tlo", [KC, P, H], F16, kind="ExternalInput")
    w2h_d = nc.dram_tensor("w2thi", [KH, P, C], F16, kind="ExternalInput")
    w2l_d = nc.dram_tensor("w2tlo", [KH, P, C], F16, kind="ExternalInput")
    g1_d = nc.dram_tensor("g1", [H], F32, kind="ExternalInput")
    be1_d = nc.dram_tensor("be1", [H], F32, kind="ExternalInput")
    g2_d = nc.dram_tensor("g2", [C], F32, kind="ExternalInput")
    be2_d = nc.dram_tensor("be2", [C], F32, kind="ExternalInput")
    out_d = nc.dram_tensor("out", [P, KC, T, MLOC], F16,
                           kind="ExternalOutput")

    groups = [list(range(NCORES))]

    with tile.TileContext(nc) as tc:
        with (
            tc.tile_pool(name="const", bufs=1) as const,
            tc.tile_pool(name="dram", bufs=1, space="DRAM") as dram,
            tc.tile_pool(name="pv", bufs=2) as pv,
            tc.tile_pool(name="pc", bufs=3) as pc,
            tc.tile_pool(name="ps_a", bufs=3, space="PSUM") as ps_a,
            tc.tile_pool(name="ps_b", bufs=2, space="PSUM") as ps_b,
        ):
            # ---- constant bias vectors (before big loads; cheap) -------
            neg1 = const.tile([P, 1], F32, name="neg1")
            nc.vector.memset(neg1[:], -1.0)
            epsc = const.tile([P, 1], F32, name="epsc")
            nc.vector.memset(epsc[:], EPS)
            # FC2 runs on {-1,+1} spikes: y2' = 2*y2 - const, var' = 4*var.
            # Using 4*EPS keeps rsqrt(var'+4e)*2 == rsqrt(var+e) exactly, so
            # the BN2 output matches the reference bit-for-bit in real
            # arithmetic (plain EPS would shift h2 by ~4e-5 relative).
            epsc4 = const.tile([P, 1], F32, name="epsc4")
            nc.vector.memset(epsc4[:], 4.0 * EPS)

            def colvec(dst_k, src):
                t_ = const.tile([P, dst_k], F32, name=f"cv_{src.name}")
                nc.sync.dma_start(
                    t_[:], src.ap().rearrange("(a p) -> p a", p=P))
                return t_

            # ---- persistent tensors ------------------------------------
            s1 = const.tile([P, T, KH, MLOC], F16, name="s1")
            asum = const.tile([P, KH, len(A_PAIRS)], F32)
            asq = const.tile([P, KH, len(A_PAIRS)], F32)
            stg = const.tile([P, KH, 2], F32, name="stg")
            dsc1 = const.tile([P, KH], F32)
            dsh1 = const.tile([P, KH], F32)
            ar_in = [dram.tile([P, 2], F32, name=f"ari{a}") for a in range(KH)]
            ar_out = [dram.tile([P, 2], F32, name=f"aro{a}")
                      for a in range(KH)]

            # ================= phase A scope (x, w1, y1) ================
            with (
                tc.tile_pool(name="xp", bufs=1) as xp,
                tc.tile_pool(name="wp", bufs=6) as wp,
                tc.tile_pool(name="y1p", bufs=LAG + 2) as y1p,
            ):
                xh_t, xl_t = [], []
                w1h_t, w1l_t = {}, {}
                g1t = be1t = g2t = be2t = None

                def load_w1(a):
                    th = wp.tile([P, KC, P], F16, tag="w1h", name=f"w1h{a}")
                    nc.sync.dma_start(
                        th[:], w1h_d.ap()[:, :, a * P:(a + 1) * P]
                        .rearrange("k p h -> p k h"))
                    tl = wp.tile([P, KC, P], F16, tag="w1l", name=f"w1l{a}")
                    nc.sync.dma_start(
                        tl[:], w1l_d.ap()[:, :, a * P:(a + 1) * P]
                        .rearrange("k p h -> p k h"))
                    w1h_t[a] = th
                    w1l_t[a] = tl

                def load_x(ci):
                    r0, rl = A_PAIRS[ci]
                    xh = xp.tile([P, KC, rl], F16, name=f"xh{ci}")
                    nc.sync.dma_start(
                        xh[:], xh_d.ap()[:, :, r0:r0 + rl]
                        .rearrange("k p r -> p k r"))
                    xl = xp.tile([P, KC, rl], F16, name=f"xl{ci}")
                    nc.sync.dma_start(
                        xl[:], xl_d.ap()[:, :, r0:r0 + rl]
                        .rearrange("k p r -> p k r"))
                    xh_t.append(xh)
                    xl_t.append(xl)

                y1tiles = {}

                def emit_fc1(a):
                    y1t = y1p.tile([P, R], F32, tag="y1", name=f"y1_{a}")
                    y1tiles[a] = y1t
                    for ci, (r0, rl) in enumerate(A_PAIRS):
                        ps = ps_a.tile([P, 1024], F32, tag="mmA")
                        for sub in range(0, rl, 512):
                            n = min(512, rl - sub)
                            idx = 0
                            for wt, xt in ((w1h_t[a], xh_t[ci]),
                                           (w1l_t[a], xh_t[ci]),
                                           (w1h_t[a], xl_t[ci])):
                                for k in range(KC):
                                    nc.tensor.matmul(
                                        ps[:, sub:sub + n],
                                        wt[:, k, :],
                                        xt[:, k, sub:sub + n],
                                        start=(idx == 0),
                                        stop=(idx == 8),
                                    )
                                    idx += 1
                        nc.scalar.activation(
                            y1t[:, r0:r0 + rl], ps[:, :rl], AF.Identity,
                            accum_out=asum[:, a, ci:ci + 1])
                        nc.scalar.activation(
                            ps[:, :rl], ps[:, :rl], AF.Square,
                            accum_out=asq[:, a, ci:ci + 1])
                    # per-htile stats -> AllReduce (gpsimd queue)
                    st = pc.tile([P, 2], F32, tag="st", name=f"st{a}")
                    nc.vector.tensor_reduce(st[:, 0:1], asum[:, a, :], AX.X,
                                            OP.add)
                    nc.vector.tensor_reduce(st[:, 1:2], asq[:, a, :], AX.X,
                                            OP.add)
                    nc.gpsimd.dma_start(ar_in[a][:], st[:])
                    nc.gpsimd.collective_compute(
                        "AllReduce", OP.add, replica_groups=groups,
                        ins=[ar_in[a].opt()], outs=[ar_out[a].opt()],
                    )
                    nc.gpsimd.dma_start(stg[:, a, :], ar_out[a][:])

                def emit_lif1(a):
                    # BN coefficients for htile a
                    mean = pc.tile([P, 1], F32, tag="mean")
                    nc.scalar.mul(mean[:], stg[:, a, 0:1], 1.0 / NTOT)
                    e2 = pc.tile([P, 1], F32, tag="e2")
                    nc.scalar.mul(e2[:], stg[:, a, 1:2], 1.0 / NTOT)
                    msq = pc.tile([P, 1], F32, tag="msq")
                    nc.scalar.square(msq[:], mean[:])
                    var = pc.tile([P, 1], F32, tag="var")
                    nc.vector.scalar_tensor_tensor(
                        var[:], msq[:], -1.0, e2[:], OP.mult, OP.add)
                    std = pc.tile([P, 1], F32, tag="std")
                    nc.scalar.activation(std[:], var[:], AF.Sqrt,
                                         bias=epsc[:])
                    rstd = pc.tile([P, 1], F32, tag="rstd")
                    nc.vector.reciprocal(rstd[:], std[:])
                    gr = pc.tile([P, 1], F32, tag="gr")
                    nc.vector.tensor_tensor(gr[:], rstd[:], g1t[:, a:a + 1],
                                            OP.mult)
                    nc.vector.tensor_scalar(dsc1[:, a:a + 1], gr[:], 0.5,
                                            None, OP.mult)
                    mdsc = pc.tile([P, 1], F32, tag="mdsc")
                    nc.vector.tensor_tensor(mdsc[:], mean[:],
                                            dsc1[:, a:a + 1], OP.mult)
                    nc.vector.scalar_tensor_tensor(
                        dsh1[:, a:a + 1], be1t[:, a:a + 1], 0.5, mdsc[:],
                        OP.mult, OP.subtract)
                    # BN affine in place on DVE: d = y*dsc + dsh
                    y1t = y1tiles.pop(a)
                    nc.vector.tensor_scalar(
                        y1t[:], y1t[:], dsc1[:, a:a + 1], dsh1[:, a:a + 1],
                        OP.mult, OP.add)
                    # sequential LIF; h_t overwrites d_t in place
                    v = pv.tile([P, MLOC], F32, tag="v")
                    d0 = y1t[:, 0:MLOC]
                    nc.vector.scalar_tensor_tensor(v[:], d0, 1.0, d0,
                                                   OP.is_lt, OP.mult)
                    for t in range(1, T):
                        dt = y1t[:, t * MLOC:(t + 1) * MLOC]
                        nc.vector.scalar_tensor_tensor(dt, v[:], 0.5, dt,
                                                       OP.mult, OP.add)
                        if t < T - 1:
                            v = pv.tile([P, MLOC], F32, tag="v")
                            nc.vector.scalar_tensor_tensor(
                                v[:], dt, 1.0, dt, OP.is_lt, OP.mult)
                    # spikes: s' = Sign(h - 1) in {-1, +1}, all T at once
                    hv = y1t[:].rearrange("p (t m) -> p t m", t=T)
                    nc.scalar.activation(s1[:, :, a, :], hv, AF.Sign,
                                         bias=neg1[:])

                # ---- FC1 + pipelined stats/LIF1 (interleaved loads) ----
                load_w1(0)
                load_x(0)
                load_x(1)
                g1t, be1t = colvec(KH, g1_d), colvec(KH, be1_d)
                g2t, be2t = colvec(KC, g2_d), colvec(KC, be2_d)
                load_w1(1)
                load_x(2)
                load_w1(2)
                load_x(3)
                for a in range(3, KH):
                    load_w1(a)

                for a in range(KH):
                    emit_fc1(a)
                    if a >= LAG:
                        emit_lif1(a - LAG)
                for a in range(KH - LAG, KH):
                    emit_lif1(a)

            # ================= phase B scope (y2, LIF2) =================
            with (
                tc.tile_pool(name="bp", bufs=1) as bp,
                tc.tile_pool(name="pv2", bufs=2) as pv2,
            ):
                w2h = bp.tile([P, KH, C], F16, name="w2h")
                nc.sync.dma_start(w2h[:],
                                  w2h_d.ap().rearrange("k p c -> p k c"))
                w2l = bp.tile([P, KH, C], F16, name="w2l")
                nc.sync.dma_start(w2l[:],
                                  w2l_d.ap().rearrange("k p c -> p k c"))
                y2r = bp.tile([P, KC, T, MLOC], F32, name="y2r")
                nb2 = len(B_CHUNKS) * T
                asum2 = const.tile([P, KC, nb2], F32)
                asq2 = const.tile([P, KC, nb2], F32)
                for mi, (m0, mlen) in enumerate(B_CHUNKS):
                    for t in range(T):
                        for ct in range(KC):
                            ps2 = ps_b.tile([P, 392], F32, tag="mm2")
                            idx = 0
                            for k in range(KH):
                                for wsp in (w2h, w2l):
                                    nc.tensor.matmul(
                                        ps2[:, :mlen],
                                        wsp[:, k, ct * P:(ct + 1) * P],
                                        s1[:, t, k, m0:m0 + mlen],
                                        start=(idx == 0),
                                        stop=(idx == 2 * KH - 1),
                                    )
                                    idx += 1
                            gi = mi * T + t
                            nc.scalar.activation(
                                y2r[:, ct, t, m0:m0 + mlen], ps2[:, :mlen],
                                AF.Identity,
                                accum_out=asum2[:, ct, gi:gi + 1])
                            nc.scalar.activation(
                                ps2[:, :mlen], ps2[:, :mlen], AF.Square,
                                accum_out=asq2[:, ct, gi:gi + 1])

                # ---- BN2 stat allreduce --------------------------------
                st2 = pc.tile([P, 2 * KC], F32, name="st2")
                nc.vector.tensor_reduce(st2[:, 0:KC], asum2[:], AX.X, OP.add)
                nc.vector.tensor_reduce(st2[:, KC:2 * KC], asq2[:], AX.X,
                                        OP.add)
                st2_in = dram.tile([P, 2 * KC], F32)
                st2_out = dram.tile([P, 2 * KC], F32)
                nc.gpsimd.dma_start(st2_in[:], st2[:])
                nc.gpsimd.collective_compute(
                    "AllReduce", OP.add, replica_groups=groups,
                    ins=[st2_in.opt()], outs=[st2_out.opt()],
                )
                stg2 = pc.tile([P, 2 * KC], F32, name="stg2")
                nc.gpsimd.dma_start(stg2[:], st2_out[:])

                mean2 = pc.tile([P, KC], F32)
                nc.scalar.mul(mean2[:], stg2[:, 0:KC], 1.0 / NTOT)
                e22 = pc.tile([P, KC], F32)
                nc.scalar.mul(e22[:], stg2[:, KC:2 * KC], 1.0 / NTOT)
                msq2 = pc.tile([P, KC], F32)
                nc.scalar.square(msq2[:], mean2[:])
                var2 = pc.tile([P, KC], F32)
                nc.vector.scalar_tensor_tensor(
                    var2[:], msq2[:], -1.0, e22[:], OP.mult, OP.add)
                std2 = pc.tile([P, KC], F32)
                nc.scalar.activation(std2[:], var2[:], AF.Sqrt, bias=epsc4[:])
                rstd2 = pc.tile([P, KC], F32)
                nc.vector.reciprocal(rstd2[:], std2[:])
                gr2 = pc.tile([P, KC], F32)
                nc.vector.tensor_tensor(gr2[:], rstd2[:], g2t[:], OP.mult)
                dsc2 = pc.tile([P, KC], F32)
                nc.vector.tensor_scalar(dsc2[:], gr2[:], 0.5, None, OP.mult)
                mdsc2 = pc.tile([P, KC], F32)
                nc.vector.tensor_tensor(mdsc2[:], mean2[:], dsc2[:], OP.mult)
                dsh2 = pc.tile([P, KC], F32)
                nc.vector.scalar_tensor_tensor(
                    dsh2[:], be2t[:], 0.5, mdsc2[:], OP.mult, OP.subtract)

                # ---- LIF2 + output -------------------------------------
                for ct in range(KC):
                    nc.vector.tensor_scalar(
                        y2r[:, ct, :, :], y2r[:, ct, :, :],
                        dsc2[:, ct:ct + 1], dsh2[:, ct:ct + 1],
                        OP.mult, OP.add)
                v2 = pv2.tile([P, KC, MLOC], F32, tag="v2")
                d0 = y2r[:, :, 0, :]
                nc.vector.scalar_tensor_tensor(v2[:], d0, 1.0, d0,
                                               OP.is_lt, OP.mult)
                for t in range(T):
                    if t > 0:
                        dt = y2r[:, :, t, :]
                        nc.vector.scalar_tensor_tensor(dt, v2[:], 0.5, dt,
                                                       OP.mult, OP.add)
                        if t < T - 1:
                            v2 = pv2.tile([P, KC, MLOC], F32, tag="v2")
                            nc.vector.scalar_tensor_tensor(
                                v2[:], dt, 1.0, dt, OP.is_lt, OP.mult)
                    s2 = pv2.tile([P, KC, MLOC], F16, tag="s2")
                    nc.scalar.activation(s2[:], y2r[:, :, t, :], AF.Sign,
                                         bias=neg1[:])
                    nc.sync.dma_start(out_d.ap()[:, :, t, :], s2[:])

    nc.compile()
    return nc


_NC = None
TRACE = False          # set by test harness to capture an NTFF profile
LAST_RESULT = None     # BassKernelResults of the most recent run


def _get_nc():
    global _NC
    if _NC is None:
        _NC = _build()
    return _NC


def _split_f16(a):
    hi = a.astype(np.float16)
    lo = (a - hi.astype(np.float32)).astype(np.float16)
    return np.ascontiguousarray(hi), np.ascontiguousarray(lo)


def _in_maps(x, W1, b1, g1, be1, W2, b2, g2, be2):
    x = np.asarray(x, dtype=np.float32)
    w1t = np.asarray(W1, np.float32).T.reshape(KC, P, H)
    w1thi, w1tlo = _split_f16(w1t)
    w2t = np.asarray(W2, np.float32).T.reshape(KH, P, C)
    w2thi, w2tlo = _split_f16(w2t)
    shared = {
        "w1thi": w1thi, "w1tlo": w1tlo,
        "w2thi": w2thi, "w2tlo": w2tlo,
        "g1": np.asarray(g1, np.float32),
        "be1": np.asarray(be1, np.float32),
        "g2": np.asarray(g2, np.float32),
        "be2": np.asarray(be2, np.float32),
    }
    in_maps = []
    for i in range(NCORES):
        xt = x[:, i * BLOC:(i + 1) * BLOC].reshape(R, C).T.reshape(KC, P, R)
        xthi, xtlo = _split_f16(xt)
        in_maps.append({"xthi": xthi, "xtlo": xtlo, **shared})
    return in_maps


def kernel(x, W1, b1, g1, be1, W2, b2, g2, be2):
    nc = _get_nc()
    in_maps = _in_maps(x, W1, b1, g1, be1, W2, b2, g2, be2)
    res = run_bass_kernel_spmd(nc, in_maps, core_ids=list(range(NCORES)),
                               trace=TRACE)
    global LAST_RESULT
    LAST_RESULT = res
    parts = []
    for i in range(NCORES):
        arr = res.results[i]["out"]              # [P, KC, T, MLOC] f16 +-1
        s = (arr.astype(np.float32) + 1.0) * 0.5
        parts.append(s.transpose(2, 3, 1, 0).reshape(T, BLOC, NN, C))
    return np.concatenate(parts, axis=1)


# revision 15
# speedup vs baseline: 1.3415x; 1.0678x over previous
"""Trainium2 Bass kernel for the DEER-MLP spiking network (v2).

Network: x(4,32,196,384) -> FC1(384->1536) -> BatchNorm -> LIF(T=4) ->
FC2(1536->384) -> BatchNorm -> LIF -> spikes(4,32,196,384).

Math notes:
- 10 DEER Newton iterations over T=4 steps converge exactly to the plain
  sequential LIF recurrence; we compute that directly.
- b1/b2 are dropped: BatchNorm immediately follows each Linear and is
  invariant to per-channel shifts.
- Spikes are produced as s' = Sign(h-1) in {-1,+1} by the Scalar engine.
  FC2 runs directly on s' : y2' = W2 @ s' = 2*y2 - rowsum(W2), and BN2 is
  invariant to per-channel affine maps, so the final result is unchanged.
  The host maps the final spike train back to {0,1} via (s'+1)/2.

Distribution: data-parallel over B across 8 cores (4 batches / 784 lanes
per core).  BatchNorm statistics are the only cross-core coupling.

Pipeline (single NEFF per core):
- FC1 runs htile-outer (12 tiles of 128 H-channels x 7 R-chunks, 3-pass
  fp16 limb matmul).  Each htile's y1 stays SBUF-resident; PSUM is
  evacuated by the Scalar engine (Identity+accum for sum, Square+accum
  for sumsq, the square written back into PSUM in place).
- Per-htile [128,2] stat AllReduce rides the gpsimd queue, pipelined
  under the FC1 matmuls of later htiles (no global barrier).
- LIF1 per htile (lagged 2 htiles behind FC1): BN affine as one DVE
  tensor_scalar, h/v updates as 6 DVE scalar_tensor_tensor ops, spikes
  as one Scalar Sign activation into fp16 s1.
- FC2 (2-pass fp16) consumes s1; stats fused into PSUM evacuation; one
  [128,6] AllReduce; LIF2 on DVE; Sign + DMA out per timestep, fp16.

Host side: pre-transpose/limb-split x and W; post-transpose the fp16
+-1 spike output back to (T,B,N,C) fp32 {0,1}.
"""

import numpy as np

import concourse.bass as bass
import concourse.mybir as mybir
import concourse.tile as tile
from concourse import bacc
from concourse.bass_utils import run_bass_kernel_spmd

F32 = mybir.dt.float32
F16 = mybir.dt.float16
AF = mybir.ActivationFunctionType
OP = mybir.AluOpType
AX = mybir.AxisListType

T, B, NN, C, H = 4, 32, 196, 384, 1536
NCORES = 8
BLOC = B // NCORES            # 4 batches per core
MLOC = BLOC * NN              # 784 lanes per core
R = T * MLOC                  # 3136 flattened (t, m) rows per core
NTOT = T * B * NN             # 25088 batchnorm samples per channel
KC = C // 128                 # 3 c-tiles
KH = H // 128                 # 12 h-tiles
EPS = 1e-5
P = 128
LAG = 4                       # htiles between FC1 and its LIF emission

# FC1 R-chunk pairs: PSUM tiles of [128, 1024] (2 banks), last 64 ragged.
A_PAIRS = [(0, 1024), (1024, 1024), (2048, 1024), (3072, 64)]
# stat-AllReduce groups of htiles: pairs early, singletons at the end so
# the last groups' AR+LIF tail before FC2 is as short as possible.
GROUPS = [(0, 2), (2, 2), (4, 2), (6, 2), (8, 2), (10, 1), (11, 1)]
G_OF = {}
for _gi, (_g0, _gn) in enumerate(GROUPS):
    for _a in range(_g0, _g0 + _gn):
        G_OF[_a] = _gi
B_CHUNKS = [(0, MLOC // 2), (MLOC // 2, MLOC // 2)]


def _build():
    nc = bacc.Bacc("TRN2", target_bir_lowering=False, debug=False,
                   num_devices=NCORES)

    xh_d = nc.dram_tensor("xthi", [KC, P, R], F16, kind="ExternalInput")
    xl_d = nc.dram_tensor("xtlo", [KC, P, R], F16, kind="ExternalInput")
    w1h_d = nc.dram_tensor("w1thi", [KC, P, H], F16, kind="ExternalInput")
    w1l_d = nc.dram_tensor("w1tlo", [KC, P, H], F16, kind="ExternalInput")
    w2h_d = nc.dram_tensor("w2thi", [KH, P, C], F16, kind="ExternalInput")
    w2l_d = nc.dram_tensor("w2tlo", [KH, P, C], F16, kind="ExternalInput")
    g1_d = nc.dram_tensor("g1", [H], F32, kind="ExternalInput")
    be1_d = nc.dram_tensor("be1", [H], F32, kind="ExternalInput")
    g2_d = nc.dram_tensor("g2", [C], F32, kind="ExternalInput")
    be2_d = nc.dram_tensor("be2", [C], F32, kind="ExternalInput")
    out_d = nc.dram_tensor("out", [P, KC, T, MLOC], F16,
                           kind="ExternalOutput")

    groups = [list(range(NCORES))]

    with tile.TileContext(nc) as tc:
        with (
            tc.tile_pool(name="const", bufs=1) as const,
            tc.tile_pool(name="dram", bufs=1, space="DRAM") as dram,
            tc.tile_pool(name="pv", bufs=2) as pv,
            tc.tile_pool(name="pc", bufs=3) as pc,
        ):
            # ---- constant bias vectors (before big loads; cheap) -------
            neg1 = const.tile([P, 1], F32, name="neg1")
            nc.vector.memset(neg1[:], -1.0)
            epsc = const.tile([P, 1], F32, name="epsc")
            nc.vector.memset(epsc[:], EPS)
            # FC2 runs on {-1,+1} spikes: y2' = 2*y2 - const, var' = 4*var.
            # Using 4*EPS keeps rsqrt(var'+4e)*2 == rsqrt(var+e) exactly, so
            # the BN2 output matches the reference bit-for-bit in real
            # arithmetic (plain EPS would shift h2 by ~4e-5 relative).
            epsc4 = const.tile([P, 1], F32, name="epsc4")
            nc.vector.memset(epsc4[:], 4.0 * EPS)

            def colvec(dst_k, src):
                t_ = const.tile([P, dst_k], F32, name=f"cv_{src.name}")
                nc.sync.dma_start(
                    t_[:], src.ap().rearrange("(a p) -> p a", p=P))
                return t_

            # ---- persistent tensors ------------------------------------
            s1 = const.tile([P, T, KH, MLOC], F16, name="s1")
            asum = const.tile([P, KH, len(A_PAIRS)], F32)
            asq = const.tile([P, KH, len(A_PAIRS)], F32)
            NG = len(GROUPS)
            stp = const.tile([P, NG, 4], F32, name="stp")
            stg = const.tile([P, NG, 4], F32, name="stg")
            dsc1 = const.tile([P, KH], F32)
            dsh1 = const.tile([P, KH], F32)
            ar_in = [dram.tile([P, 4], F32, name=f"ari{p}")
                     for p in range(NG)]
            ar_out = [dram.tile([P, 4], F32, name=f"aro{p}")
                      for p in range(NG)]

            # ================= phase A scope (x, w1, y1) ================
            with (
                tc.tile_pool(name="xp", bufs=1) as xp,
                tc.tile_pool(name="wp", bufs=6) as wp,
                tc.tile_pool(name="y1p", bufs=LAG + 2) as y1p,
                tc.tile_pool(name="ps_a", bufs=3, space="PSUM") as ps_a,
            ):
                xh_t, xl_t = [], []
                w1h_t, w1l_t = {}, {}
                g1t = be1t = g2t = be2t = None

                def load_w1(a):
                    th = wp.tile([P, KC, P], F16, tag="w1h", name=f"w1h{a}")
                    nc.sync.dma_start(
                        th[:], w1h_d.ap()[:, :, a * P:(a + 1) * P]
                        .rearrange("k p h -> p k h"))
                    tl = wp.tile([P, KC, P], F16, tag="w1l", name=f"w1l{a}")
                    nc.sync.dma_start(
                        tl[:], w1l_d.ap()[:, :, a * P:(a + 1) * P]
                        .rearrange("k p h -> p k h"))
                    w1h_t[a] = th
                    w1l_t[a] = tl

                def load_x(ci):
                    r0, rl = A_PAIRS[ci]
                    xh = xp.tile([P, KC, rl], F16, name=f"xh{ci}")
                    nc.sync.dma_start(
                        xh[:], xh_d.ap()[:, :, r0:r0 + rl]
                        .rearrange("k p r -> p k r"))
                    xl = xp.tile([P, KC, rl], F16, name=f"xl{ci}")
                    nc.sync.dma_start(
                        xl[:], xl_d.ap()[:, :, r0:r0 + rl]
                        .rearrange("k p r -> p k r"))
                    xh_t.append(xh)
                    xl_t.append(xl)

                y1tiles = {}

                def emit_fc1(a):
                    y1t = y1p.tile([P, R], F32, tag="y1", name=f"y1_{a}")
                    y1tiles[a] = y1t
                    for ci, (r0, rl) in enumerate(A_PAIRS):
                        ps = ps_a.tile([P, 1024], F32, tag="mmA")
                        for sub in range(0, rl, 512):
                            n = min(512, rl - sub)
                            idx = 0
                            for wt, xt in ((w1h_t[a], xh_t[ci]),
                                           (w1l_t[a], xh_t[ci]),
                                           (w1h_t[a], xl_t[ci])):
                                for k in range(KC):
                                    nc.tensor.matmul(
                                        ps[:, sub:sub + n],
                                        wt[:, k, :],
                                        xt[:, k, sub:sub + n],
                                        start=(idx == 0),
                                        stop=(idx == 8),
                                    )
                                    idx += 1
                        nc.scalar.activation(
                            y1t[:, r0:r0 + rl], ps[:, :rl], AF.Identity,
                            accum_out=asum[:, a, ci:ci + 1])
                        nc.scalar.activation(
                            ps[:, :rl], ps[:, :rl], AF.Square,
                            accum_out=asq[:, a, ci:ci + 1])
                    # per-htile stats into the group slot; AR per group
                    pr = G_OF[a]
                    g0, gn = GROUPS[pr]
                    j = a - g0
                    nc.vector.tensor_reduce(stp[:, pr, j:j + 1],
                                            asum[:, a, :], AX.X, OP.add)
                    nc.vector.tensor_reduce(stp[:, pr, 2 + j:3 + j],
                                            asq[:, a, :], AX.X, OP.add)
                    if j == gn - 1:
                        nc.gpsimd.dma_start(ar_in[pr][:], stp[:, pr, :])
                        nc.gpsimd.collective_compute(
                            "AllReduce", OP.add, replica_groups=groups,
                            ins=[ar_in[pr].opt()], outs=[ar_out[pr].opt()],
                        )
                        nc.gpsimd.dma_start(stg[:, pr, :], ar_out[pr][:])

                def emit_lif_group(pr):
                    g0, gn = GROUPS[pr]
                    # BN coefficients for the group, batched [P, gn]
                    mean = pc.tile([P, 2], F32, tag="mean")
                    nc.scalar.mul(mean[:, :gn], stg[:, pr, 0:gn], 1.0 / NTOT)
                    e2 = pc.tile([P, 2], F32, tag="e2")
                    nc.scalar.mul(e2[:, :gn], stg[:, pr, 2:2 + gn],
                                  1.0 / NTOT)
                    msq = pc.tile([P, 2], F32, tag="msq")
                    nc.scalar.square(msq[:, :gn], mean[:, :gn])
                    var = pc.tile([P, 2], F32, tag="var")
                    nc.vector.scalar_tensor_tensor(
                        var[:, :gn], msq[:, :gn], -1.0, e2[:, :gn],
                        OP.mult, OP.add)
                    std = pc.tile([P, 2], F32, tag="std")
                    nc.scalar.activation(std[:, :gn], var[:, :gn], AF.Sqrt,
                                         bias=epsc[:])
                    rstd = pc.tile([P, 2], F32, tag="rstd")
                    nc.vector.reciprocal(rstd[:, :gn], std[:, :gn])
                    gr = pc.tile([P, 2], F32, tag="gr")
                    nc.vector.tensor_tensor(gr[:, :gn], rstd[:, :gn],
                                            g1t[:, g0:g0 + gn], OP.mult)
                    nc.vector.tensor_scalar(dsc1[:, g0:g0 + gn], gr[:, :gn],
                                            0.5, None, OP.mult)
                    mdsc = pc.tile([P, 2], F32, tag="mdsc")
                    nc.vector.tensor_tensor(mdsc[:, :gn], mean[:, :gn],
                                            dsc1[:, g0:g0 + gn], OP.mult)
                    nc.vector.scalar_tensor_tensor(
                        dsh1[:, g0:g0 + gn], be1t[:, g0:g0 + gn], 0.5,
                        mdsc[:, :gn], OP.mult, OP.subtract)
                    for a in range(g0, g0 + gn):
                        emit_lif1(a)

                def emit_lif1(a):
                    # BN affine in place on DVE: d = y*dsc + dsh
                    y1t = y1tiles.pop(a)
                    nc.vector.tensor_scalar(
                        y1t[:], y1t[:], dsc1[:, a:a + 1], dsh1[:, a:a + 1],
                        OP.mult, OP.add)
                    # sequential LIF; h_t overwrites d_t in place
                    v = pv.tile([P, MLOC], F32, tag="v")
                    d0 = y1t[:, 0:MLOC]
                    nc.vector.scalar_tensor_tensor(v[:], d0, 1.0, d0,
                                                   OP.is_lt, OP.mult)
                    for t in range(1, T):
                        dt = y1t[:, t * MLOC:(t + 1) * MLOC]
                        nc.vector.scalar_tensor_tensor(dt, v[:], 0.5, dt,
                                                       OP.mult, OP.add)
                        if t < T - 1:
                            v = pv.tile([P, MLOC], F32, tag="v")
                            nc.vector.scalar_tensor_tensor(
                                v[:], dt, 1.0, dt, OP.is_lt, OP.mult)
                    # spikes: s' = Sign(h - 1) in {-1, +1}, all T at once
                    hv = y1t[:].rearrange("p (t m) -> p t m", t=T)
                    nc.scalar.activation(s1[:, :, a, :], hv, AF.Sign,
                                         bias=neg1[:])

                # ---- FC1 + pipelined stats/LIF1 (interleaved loads) ----
                load_w1(0)
                load_x(0)
                load_x(1)
                g1t, be1t = colvec(KH, g1_d), colvec(KH, be1_d)
                g2t, be2t = colvec(KC, g2_d), colvec(KC, be2_d)
                load_w1(1)
                load_x(2)
                load_w1(2)
                load_x(3)
                for a in range(3, KH):
                    load_w1(a)

                LAGP = 2              # groups of lag between FC1 and LIF
                done_groups = 0
                for a in range(KH):
                    emit_fc1(a)
                    pr = G_OF[a]
                    g0, gn = GROUPS[pr]
                    if a == g0 + gn - 1:
                        done_groups += 1
                        if done_groups - LAGP >= 1:
                            emit_lif_group(done_groups - LAGP - 1)
                for pr in range(max(0, done_groups - LAGP), len(GROUPS)):
                    emit_lif_group(pr)

            # ================= phase B scope (y2, LIF2) =================
            with (
                tc.tile_pool(name="bp", bufs=1) as bp,
                tc.tile_pool(name="pv2", bufs=2) as pv2,
                tc.tile_pool(name="ps_b", bufs=6, space="PSUM") as ps_b,
            ):
                w2h = bp.tile([P, KH, C], F16, name="w2h")
                nc.sync.dma_start(w2h[:],
                                  w2h_d.ap().rearrange("k p c -> p k c"))
                w2l = bp.tile([P, KH, C], F16, name="w2l")
                nc.sync.dma_start(w2l[:],
                                  w2l_d.ap().rearrange("k p c -> p k c"))
                y2r = bp.tile([P, KC, T, MLOC], F32, name="y2r")
                nb2 = len(B_CHUNKS) * T
                asum2 = const.tile([P, KC, nb2], F32)
                asq2 = const.tile([P, KC, nb2], F32)
                def fc2_mms(ps2, mi, t, ct, k_lo, k_hi):
                    m0, mlen = B_CHUNKS[mi]
                    for k in range(k_lo, k_hi + 1):
                        for li, wsp in enumerate((w2h, w2l)):
                            nc.tensor.matmul(
                                ps2[:, :mlen],
                                wsp[:, k, ct * P:(ct + 1) * P],
                                s1[:, t, k, m0:m0 + mlen],
                                start=(k == 0 and li == 0),
                                stop=(k == KH - 1 and li == 1),
                            )

                def fc2_evac(ps2, mi, t, ct):
                    m0, mlen = B_CHUNKS[mi]
                    gi = mi * T + t
                    nc.scalar.activation(
                        y2r[:, ct, t, m0:m0 + mlen], ps2[:, :mlen],
                        AF.Identity, accum_out=asum2[:, ct, gi:gi + 1])
                    nc.scalar.activation(
                        ps2[:, :mlen], ps2[:, :mlen], AF.Square,
                        accum_out=asq2[:, ct, gi:gi + 1])

                fc2_groups = [(mi, t, ct)
                              for mi in range(len(B_CHUNKS))
                              for t in range(T)
                              for ct in range(KC)]
                # Partial contraction: open the first 6 groups with the
                # early htiles (spikes ready long before FC1's tail), then
                # finish k=8..11 as the late htiles' spikes land.  Fills
                # the last-AR + LIF tail with real PE work.
                NPART = 6
                part = []
                for (mi, t, ct) in fc2_groups[:NPART]:
                    ps2 = ps_b.tile([P, 392], F32, tag="mm2")
                    fc2_mms(ps2, mi, t, ct, 0, 7)
                    part.append(ps2)
                for k_lo, k_hi in ((8, 9), (10, 10), (11, 11)):
                    for gi, (mi, t, ct) in enumerate(fc2_groups[:NPART]):
                        fc2_mms(part[gi], mi, t, ct, k_lo, k_hi)
                for gi, (mi, t, ct) in enumerate(fc2_groups[:NPART]):
                    fc2_evac(part[gi], mi, t, ct)
                for (mi, t, ct) in fc2_groups[NPART:]:
                    ps2 = ps_b.tile([P, 392], F32, tag="mm2")
                    fc2_mms(ps2, mi, t, ct, 0, KH - 1)
                    fc2_evac(ps2, mi, t, ct)

                # ---- BN2 stat allreduce --------------------------------
                st2 = pc.tile([P, 2 * KC], F32, name="st2")
                nc.vector.tensor_reduce(st2[:, 0:KC], asum2[:], AX.X, OP.add)
                nc.vector.tensor_reduce(st2[:, KC:2 * KC], asq2[:], AX.X,
                                        OP.add)
                st2_in = dram.tile([P, 2 * KC], F32)
                st2_out = dram.tile([P, 2 * KC], F32)
                nc.gpsimd.dma_start(st2_in[:], st2[:])
                nc.gpsimd.collective_compute(
                    "AllReduce", OP.add, replica_groups=groups,
                    ins=[st2_in.opt()], outs=[st2_out.opt()],
                )
                stg2 = pc.tile([P, 2 * KC], F32, name="stg2")
                nc.gpsimd.dma_start(stg2[:], st2_out[:])

                mean2 = pc.tile([P, KC], F32)
                nc.scalar.mul(mean2[:], stg2[:, 0:KC], 1.0 / NTOT)
                e22 = pc.tile([P, KC], F32)
                nc.scalar.mul(e22[:], stg2[:, KC:2 * KC], 1.0 / NTOT)
                msq2 = pc.tile([P, KC], F32)
                nc.scalar.square(msq2[:], mean2[:])
                var2 = pc.tile([P, KC], F32)
                nc.vector.scalar_tensor_tensor(
                    var2[:], msq2[:], -1.0, e22[:], OP.mult, OP.add)
                std2 = pc.tile([P, KC], F32)
                nc.scalar.activation(std2[:], var2[:], AF.Sqrt, bias=epsc4[:])
                rstd2 = pc.tile([P, KC], F32)
                nc.vector.reciprocal(rstd2[:], std2[:])
                gr2 = pc.tile([P, KC], F32)
                nc.vector.tensor_tensor(gr2[:], rstd2[:], g2t[:], OP.mult)
                dsc2 = pc.tile([P, KC], F32)
                nc.vector.tensor_scalar(dsc2[:], gr2[:], 0.5, None, OP.mult)
                mdsc2 = pc.tile([P, KC], F32)
                nc.vector.tensor_tensor(mdsc2[:], mean2[:], dsc2[:], OP.mult)
                dsh2 = pc.tile([P, KC], F32)
                nc.vector.scalar_tensor_tensor(
                    dsh2[:], be2t[:], 0.5, mdsc2[:], OP.mult, OP.subtract)

                # ---- LIF2 + output -------------------------------------
                for ct in range(KC):
                    nc.vector.tensor_scalar(
                        y2r[:, ct, :, :], y2r[:, ct, :, :],
                        dsc2[:, ct:ct + 1], dsh2[:, ct:ct + 1],
                        OP.mult, OP.add)
                v2 = pv2.tile([P, KC, MLOC], F32, tag="v2")
                d0 = y2r[:, :, 0, :]
                nc.vector.scalar_tensor_tensor(v2[:], d0, 1.0, d0,
                                               OP.is_lt, OP.mult)
                for t in range(T):
                    if t > 0:
                        dt = y2r[:, :, t, :]
                        nc.vector.scalar_tensor_tensor(dt, v2[:], 0.5, dt,
                                                       OP.mult, OP.add)
                        if t < T - 1:
                            v2 = pv2.tile([P, KC, MLOC], F32, tag="v2")
                            nc.vector.scalar_tensor_tensor(
                                v2[:], dt, 1.0, dt, OP.is_lt, OP.mult)
                    s2 = pv2.tile([P, KC, MLOC], F16, tag="s2")
                    nc.scalar.activation(s2[:], y2r[:, :, t, :], AF.Sign,
                                         bias=neg1[:])
                    nc.sync.dma_start(out_d.ap()[:, :, t, :], s2[:])

    nc.compile()
    return nc


_NC = None
TRACE = False          # set by test harness to capture an NTFF profile
LAST_RESULT = None     # BassKernelResults of the most recent run


def _get_nc():
    global _NC
    if _NC is None:
        _NC = _build()
    return _NC


def _split_f16(a):
    hi = a.astype(np.float16)
    lo = (a - hi.astype(np.float32)).astype(np.float16)
    return np.ascontiguousarray(hi), np.ascontiguousarray(lo)


def _in_maps(x, W1, b1, g1, be1, W2, b2, g2, be2):
    x = np.asarray(x, dtype=np.float32)
    w1t = np.asarray(W1, np.float32).T.reshape(KC, P, H)
    w1thi, w1tlo = _split_f16(w1t)
    w2t = np.asarray(W2, np.float32).T.reshape(KH, P, C)
    w2thi, w2tlo = _split_f16(w2t)
    shared = {
        "w1thi": w1thi, "w1tlo": w1tlo,
        "w2thi": w2thi, "w2tlo": w2tlo,
        "g1": np.asarray(g1, np.float32),
        "be1": np.asarray(be1, np.float32),
        "g2": np.asarray(g2, np.float32),
        "be2": np.asarray(be2, np.float32),
    }
    in_maps = []
    for i in range(NCORES):
        xt = x[:, i * BLOC:(i + 1) * BLOC].reshape(R, C).T.reshape(KC, P, R)
        xthi, xtlo = _split_f16(xt)
        in_maps.append({"xthi": xthi, "xtlo": xtlo, **shared})
    return in_maps


def kernel(x, W1, b1, g1, be1, W2, b2, g2, be2):
    nc = _get_nc()
    in_maps = _in_maps(x, W1, b1, g1, be1, W2, b2, g2, be2)
    res = run_bass_kernel_spmd(nc, in_maps, core_ids=list(range(NCORES)),
                               trace=TRACE)
    global LAST_RESULT
    LAST_RESULT = res
    parts = []
    for i in range(NCORES):
        arr = res.results[i]["out"]              # [P, KC, T, MLOC] f16 +-1
        s = (arr.astype(np.float32) + 1.0) * 0.5
        parts.append(s.transpose(2, 3, 1, 0).reshape(T, BLOC, NN, C))
    return np.concatenate(parts, axis=1)
